# revision 1
# baseline (speedup 1.0000x reference)
"""ChildSum TreeLSTM on 8 Trainium2 NeuronCores (Bass/Tile) — v2.

Differences vs v1 baseline:
  - Zero per-level DMA: contribution rows ([h || f*c], bf16) live in one
    persistent SBUF tensor; segment-sums read them directly as matmul
    operands (children of each (level,piece) are contiguous runs because
    slots are sorted by (parent_level, parent_slot)).
  - hs^T is produced feature-major straight from the segsum matmuls, so the
    iou GEMM needs no PE transpose on the critical path.
  - x-side pre-activations are accumulated directly in the same PSUM bank
    that the recurrent iou/wfh matmuls target (no xpre materialization).
  - All input DMAs are batched up-front; outputs leave in 2 DMAs at the end.
"""

import numpy as np
from contextlib import ExitStack

N_CORES = 8
P = 128
MD = 256
TOP_CAP = 160


# ---------------------------------------------------------------- host side


def _preprocess(parent):
    parent = np.asarray(parent, dtype=np.int64)
    N = len(parent)
    level = np.zeros(N, dtype=np.int64)
    for j in range(N - 1):
        p = parent[j]
        if level[p] <= level[j]:
            level[p] = level[j] + 1
    n_levels = int(level.max()) + 1

    cnt_ge = np.zeros(n_levels + 1, dtype=np.int64)
    for l in range(n_levels - 1, -1, -1):
        cnt_ge[l] = cnt_ge[l + 1] + int((level == l).sum())
    L0 = n_levels
    for l in range(n_levels + 1):
        if cnt_ge[l] <= TOP_CAP:
            L0 = l
            break
    assert 1 <= L0 < n_levels

    is_top = level >= L0
    top_ids = np.arange(N)[is_top][np.argsort(level[is_top], kind="stable")]

    # crown slot layout: levels packed back-to-back within 128-tiles
    TNL = [int((level[top_ids] == l).sum()) for l in range(L0, n_levels)]
    TLB = []
    b = 0
    for nl in TNL:
        assert nl <= 128
        if b % P + nl > P:
            b = (b // P + 1) * P
        TLB.append(b)
        b += nl
    TSLOTS = (b + P - 1) // P * P
    TPP = TSLOTS // P
    tslot = np.full(N, -1, dtype=np.int64)
    for li, l in enumerate(range(L0, n_levels)):
        sel = top_ids[level[top_ids] == l]
        tslot[sel] = TLB[li] + np.arange(len(sel))

    # subtree partition of the bottom (bin-pack complete subtrees)
    anchor = np.full(N, -1, dtype=np.int64)
    for j in range(N - 1, -1, -1):
        if is_top[j]:
            continue
        p = parent[j]
        anchor[j] = j if (p == N or is_top[p]) else anchor[p]
    roots = np.unique(anchor[anchor >= 0])
    sizes = np.zeros(N, dtype=np.int64)
    np.add.at(sizes, anchor[anchor >= 0], 1)
    order = roots[np.argsort(-sizes[roots], kind="stable")]
    load = np.zeros(N_CORES, dtype=np.int64)
    core_of_root = {}
    for r in order:
        c = int(np.argmin(load))
        core_of_root[int(r)] = c
        load[c] += sizes[r]
    core = np.full(N, -1, dtype=np.int64)
    bot = anchor >= 0
    core[bot] = [core_of_root[int(a)] for a in anchor[bot]]

    # 128-aligned per-level regions, shared across cores
    nreal = np.zeros((N_CORES, L0), dtype=np.int64)
    for c in range(N_CORES):
        for l in range(L0):
            nreal[c, l] = int(((core == c) & (level == l)).sum())
    NLp = [int(-(-nreal[:, l].max() // P) * P) for l in range(L0)]
    LBp = np.concatenate([[0], np.cumsum(NLp)]).astype(np.int64)
    NBOT = int(LBp[L0])
    NT = NBOT // P
    NPHA = NBOT + TSLOTS

    # slot assignment: levels descending so parent slots are known; within a
    # (core, level) sort children by (parent_level, parent_slot)
    slot_of = np.full(N, -1, dtype=np.int64)
    node_at = [np.full(NPHA, -1, dtype=np.int64) for _ in range(N_CORES)]
    for j in top_ids:
        for c in range(N_CORES):
            node_at[c][NBOT + tslot[j]] = j
    for l in range(L0 - 1, -1, -1):
        for c in range(N_CORES):
            nodes = np.arange(N)[(core == c) & (level == l)]
            keys = []
            for j in nodes:
                p = int(parent[j])
                if is_top[p]:
                    keys.append((level[p], int(tslot[p]), j))
                else:
                    keys.append((level[p], int(slot_of[p]), j))
            nodes = nodes[np.lexsort(([k[2] for k in keys],
                                      [k[1] for k in keys],
                                      [k[0] for k in keys]))]
            for i, j in enumerate(nodes):
                slot_of[j] = LBp[l] + i
                node_at[c][LBp[l] + i] = j

    # ---- bottom S entries: per (level, piece) a list of source tiles ----
    # key (l, piece) -> {src_tile: S[N_CORES, P, P]}
    ent = {}
    for c in range(N_CORES):
        for j in np.arange(N)[(core == c)]:
            p = int(parent[j])
            if p == N or is_top[p]:
                continue
            ps = int(slot_of[p])
            lp = int(level[p])
            piece = (ps - LBp[lp]) // P
            t = int(slot_of[j]) // P
            key = (lp, int(piece))
            d = ent.setdefault(key, {})
            S = d.setdefault(t, np.zeros((N_CORES, P, P), np.float32))
            S[c, int(slot_of[j]) % P, ps - (LBp[lp] + piece * P)] = 1.0

    tile_level = np.zeros(NT, dtype=np.int64)  # level owning each tile
    for l in range(L0):
        tile_level[LBp[l] // P: LBp[l + 1] // P] = l

    S_list = []          # per-core stacked later
    pieces_bot = []      # (l, piece, s0, [(src_tile, sidx), ...])
    for l in range(1, L0):
        for piece in range(NLp[l] // P):
            key = (l, piece)
            srcs = []
            if key in ent:
                # old sources first, hot (level l-1) last
                for t in sorted(ent[key], key=lambda t: (tile_level[t], t)):
                    srcs.append((t, len(S_list)))
                    S_list.append(ent[key][t])
            pieces_bot.append((l, piece, int(LBp[l] + piece * P), srcs))

    # ---- cut edges: bottom child -> crown parent, pre-reduced per tslot ----
    cutent = {}
    for c in range(N_CORES):
        for j in np.arange(N)[(core == c)]:
            p = int(parent[j])
            if p == N or not is_top[p]:
                continue
            ts = int(tslot[p])
            t = int(slot_of[j]) // P
            key = (ts // P, t)
            S = cutent.setdefault(key, np.zeros((N_CORES, P, P), np.float32))
            S[c, int(slot_of[j]) % P, ts % P] = 1.0
    cut_mms = []  # (tp, src_tile, sidx)
    for (tp, t) in sorted(cutent):
        cut_mms.append((tp, t, len(S_list)))
        S_list.append(cutent[(tp, t)])

    # ---- crown: extended-prefix pieces with cumulative S ----
    pieces_crown = []  # (l, tp, cnt_ext, [(src_tp, sidx), ...])
    for li, l in enumerate(range(L0, n_levels)):
        s0, cnt = TLB[li], TNL[li]
        tp = s0 // P
        base = tp * P
        cnt_ext = s0 % P + cnt
        # crown-internal edges with parent in [base, base+cnt_ext)
        d = {}
        for j in top_ids:
            p = int(parent[j])
            if p == N or not is_top[p]:
                continue
            if not (base <= tslot[p] < base + cnt_ext):
                continue
            st = int(tslot[j]) // P
            S = d.setdefault(st, np.zeros((N_CORES, P, P), np.float32))
            S[:, int(tslot[j]) % P, int(tslot[p]) - base] = 1.0
        srcs = []
        for st in sorted(d):
            srcs.append((st, len(S_list)))
            S_list.append(d[st])
        pieces_crown.append((l, tp, int(cnt_ext), srcs))

    NS = len(S_list)
    S_all = (np.stack(S_list, 1) if NS else np.zeros((N_CORES, 1, P, P), np.float32))
    # layout [N_CORES, 128, NS*128]: tile sidx at cols [sidx*128, ...)
    S_flat = np.ascontiguousarray(S_all.transpose(0, 2, 1, 3).reshape(N_CORES, P, max(NS, 1) * P))

    meta = dict(
        N=N, L0=L0, n_levels=n_levels, level=level, parent=parent,
        is_top=is_top, tslot=tslot, top_ids=top_ids, core=core,
        TNL=TNL, TLB=TLB, TSLOTS=TSLOTS, TPP=TPP,
        NLp=NLp, LBp=LBp, NBOT=NBOT, NT=NT, NPHA=NPHA,
        slot_of=slot_of, node_at=node_at,
        pieces_bot=pieces_bot, pieces_crown=pieces_crown, cut_mms=cut_mms,
        NS=NS,
    )
    return meta, dict(S_flat=S_flat)


def _build_inputs(meta, data, embs, Wx, bx, Wh, bh, Wfh, bfh, dtypes=np.float32):
    N = meta["N"]
    NPHA = meta["NPHA"]
    IN = embs.shape[1]
    parent = meta["parent"]
    K1 = IN + 1
    KP = -(-K1 // P) * P  # padded contraction rows

    WxI = np.zeros((KP, 768), dtype=np.float32)
    WxI[:IN] = Wx[:, :768]
    WxI[IN] = bx[:768] + bh
    WxF = np.zeros((KP, 256), dtype=np.float32)
    WxF[:IN] = Wx[:, 768:1024]
    WxF[IN] = bx[768:1024] + bfh
    Whp = np.zeros((2 * P, 768), dtype=np.float32)
    Whp[:MD] = Wh
    Wfhp = np.zeros((2 * P, 256), dtype=np.float32)
    Wfhp[:MD] = Wfh

    embs_pad = np.concatenate([embs, np.zeros((1, IN), np.float32)], 0)
    in_maps = []
    for c in range(N_CORES):
        na = meta["node_at"][c]
        sel = np.where(na >= 0, na, N)
        par = np.where(na >= 0, parent[np.clip(na, 0, N - 1)], N)
        par = np.minimum(par, N)
        eT = np.zeros((KP, NPHA), dtype=np.float32)
        eT[:IN] = embs_pad[sel].T
        eT[IN] = 1.0
        pT = np.zeros((KP, NPHA), dtype=np.float32)
        pT[:IN] = embs_pad[par].T
        pT[IN] = 1.0
        import ml_dtypes
        bf = ml_dtypes.bfloat16
        in_maps.append({
            "embsT": np.ascontiguousarray(eT.astype(bf)),
            "embsparT": np.ascontiguousarray(pT.astype(bf)),
            "WxI": WxI.astype(bf), "WxF": WxF.astype(bf),
            "Whp": np.ascontiguousarray(Whp.astype(bf)),
            "Wfhp": np.ascontiguousarray(Wfhp.astype(bf)),
            "S_flat": np.ascontiguousarray(data["S_flat"][c].astype(bf)),
        })
    return in_maps


# ------------------------------------------------- numpy schedule validator


def simulate_schedule(meta, data, inputs):
    """Execute the exact device schedule in numpy (fp32) -> h [N, 256]."""
    def sig(x):
        return 1.0 / (1.0 + np.exp(-x))

    N = meta["N"]
    NT = meta["NT"]
    TPP = meta["TPP"]
    NBOT = meta["NBOT"]
    L0 = meta["L0"]
    in_maps = _build_inputs(meta, data, **inputs_to_args(inputs))
    S_flat = data["S_flat"]

    h_out = np.zeros((N, MD), np.float32)
    topc_final = None
    cc_sum = np.zeros((TPP * P, 512), np.float32)
    contribs = []
    big_save = []

    for c in range(N_CORES):
        m = in_maps[c]
        KP = m["embsT"].shape[0]
        contrib = np.zeros((NT * P, 512), np.float32)  # [h || fc] rows
        # phase A for any piece: slots [s0, s0+128)
        def phase_a(sA, npha_base=0):
            e = m["embsT"][:, sA:sA + P].astype(np.float32)
            ep = m["embsparT"][:, sA:sA + P].astype(np.float32)
            big = np.zeros((P, 1024), np.float32)
            big[:, 0:768] = e.T @ m["WxI"].astype(np.float32)
            big[:, 768:1024] = ep.T @ m["WxF"].astype(np.float32)
            return big

        def S_tile(sidx):
            return S_flat[c][:, sidx * P:(sidx + 1) * P]  # [128, 128]

        def piece_body(big, seg_hsT, seg_fc, n, leaf):
            # big [P, 1024] with A=[0:768] preacts (+Wh*hs), B=[768:1024]
            iou = big[:n, 0:768].copy()
            if not leaf:
                hsT = seg_hsT  # [256, n]
                iou += hsT.T @ m["Whp"][:MD].astype(np.float32)
            u = np.tanh(iou[:, 512:768])
            i = sig(iou[:, 0:256])
            o = sig(iou[:, 256:512])
            cc = i * u
            if not leaf:
                cc = cc + seg_fc[:n]
            th = np.tanh(cc)
            h = o * th
            fpre = big[:n, 768:1024] + h @ m["Wfhp"][:MD].astype(np.float32)
            f = sig(fpre)
            fc = f * cc
            return h, fc

        # leaves
        for piece in range(meta["NLp"][0] // P):
            s0 = piece * P
            big = phase_a(s0)
            h, fc = piece_body(big, None, None, P, True)
            contrib[s0:s0 + P, 0:256] = h
            contrib[s0:s0 + P, 256:512] = fc
        # bottom levels
        for (l, piece, s0, srcs) in meta["pieces_bot"]:
            big = phase_a(s0)
            hsT = np.zeros((MD, P), np.float32)
            fcs = np.zeros((P, MD), np.float32)
            for (t, sidx) in srcs:
                S = S_tile(sidx)  # [128 rows(child), 128 cols(parent)]
                rows = contrib[t * P:(t + 1) * P]
                hsT += rows[:, 0:256].T @ S
                fcs += S.T @ rows[:, 256:512]
            h, fc = piece_body(big, hsT, fcs, P, False)
            contrib[s0:s0 + P, 0:256] = h
            contrib[s0:s0 + P, 256:512] = fc
        # cut pre-reduce
        cc = np.zeros((TPP * P, 512), np.float32)
        for (tp, t, sidx) in meta["cut_mms"]:
            S = S_tile(sidx)
            cc[tp * P:(tp + 1) * P] += S.T @ contrib[t * P:(t + 1) * P]
        cc_sum += cc
        contribs.append(contrib)
        big_save.append(phase_a)

        # bottom outputs
        na = meta["node_at"][c]
        for s in range(NBOT):
            if na[s] >= 0:
                h_out[na[s]] = contrib[s, 0:256]

    # crown (replicated; compute once with core-0 phase A since crown embs
    # identical on all cores)
    m = in_maps[0]
    topc = np.zeros((TPP * P, 512), np.float32)
    for (l, tp, cnt_ext, srcs) in meta["pieces_crown"]:
        base = tp * P
        big = big_save[0](NBOT + base)
        hsT = np.zeros((MD, P), np.float32)
        fcs = np.zeros((P, MD), np.float32)
        for (st, sidx) in srcs:
            S = data["S_flat"][0][:, sidx * P:(sidx + 1) * P]
            rows = topc[st * P:(st + 1) * P]
            hsT += rows[:, 0:256].T @ S
            fcs += S.T @ rows[:, 256:512]
        # cc identity contribution
        ccr = cc_sum[base:base + P]
        hsT[:, :cnt_ext] += ccr[:cnt_ext, 0:256].T
        fcs[:cnt_ext] += ccr[:cnt_ext, 256:512]
        h, fc = None, None
        h, fc = _crown_body(big, hsT, fcs, cnt_ext, m)
        topc[base:base + cnt_ext, 0:256] = h
        topc[base:base + cnt_ext, 256:512] = fc
    na0 = meta["node_at"][0]
    for j in meta["top_ids"]:
        h_out[j] = topc[meta["tslot"][j], 0:256]
    return h_out


def _crown_body(big, hsT, fcs, n, m):
    def sig(x):
        return 1.0 / (1.0 + np.exp(-x))
    iou = big[:n, 0:768] + hsT[:, :n].T @ m["Whp"][:MD].astype(np.float32)
    u = np.tanh(iou[:, 512:768])
    i = sig(iou[:, 0:256])
    o = sig(iou[:, 256:512])
    cc = i * u + fcs[:n]
    th = np.tanh(cc)
    h = o * th
    f = sig(big[:n, 768:1024] + h @ m["Wfhp"][:MD].astype(np.float32))
    return h, f * cc


def inputs_to_args(inputs):
    return dict(embs=np.asarray(inputs["embs"], np.float32),
                Wx=np.asarray(inputs["Wx"], np.float32),
                bx=np.asarray(inputs["bx"], np.float32),
                Wh=np.asarray(inputs["Wh"], np.float32),
                bh=np.asarray(inputs["bh"], np.float32),
                Wfh=np.asarray(inputs["Wfh"], np.float32),
                bfh=np.asarray(inputs["bfh"], np.float32))


# ---------------------------------------------------------------- device side


def _build_program(meta, IN, sim_no_collective=False):
    import concourse.bass as bass
    import concourse.tile as tile
    from concourse import bacc, mybir

    f32 = mybir.dt.float32
    bf16 = mybir.dt.bfloat16
    SIG = mybir.ActivationFunctionType.Sigmoid
    TANH = mybir.ActivationFunctionType.Tanh

    NPHA = meta["NPHA"]
    NBOT = meta["NBOT"]
    NT = meta["NT"]
    TPP = meta["TPP"]
    NS = max(meta["NS"], 1)
    NL0 = meta["NLp"][0]
    K1 = IN + 1
    KP = -(-K1 // P) * P
    NKT = KP // P
    tile_level = np.zeros(NT, dtype=np.int64)
    for l in range(meta["L0"]):
        tile_level[meta["LBp"][l] // P: meta["LBp"][l + 1] // P] = l

    nc = bacc.Bacc("TRN2", target_bir_lowering=False, debug=False,
                   num_devices=N_CORES)

    embsT_d = nc.dram_tensor("embsT", [KP, NPHA], bf16, kind="ExternalInput").ap()
    embsparT_d = nc.dram_tensor("embsparT", [KP, NPHA], bf16, kind="ExternalInput").ap()
    WxI_d = nc.dram_tensor("WxI", [KP, 768], bf16, kind="ExternalInput").ap()
    WxF_d = nc.dram_tensor("WxF", [KP, 256], bf16, kind="ExternalInput").ap()
    Whp_d = nc.dram_tensor("Whp", [2 * P, 768], bf16, kind="ExternalInput").ap()
    Wfhp_d = nc.dram_tensor("Wfhp", [2 * P, 256], bf16, kind="ExternalInput").ap()
    S_d = nc.dram_tensor("S_flat", [P, NS * P], bf16, kind="ExternalInput").ap()

    contrib_out = nc.dram_tensor("contrib_out", [P, NT * 512], bf16,
                                 kind="ExternalOutput").ap()
    topc_out = nc.dram_tensor("topc_out", [P, TPP * 512], bf16,
                              kind="ExternalOutput").ap()
    cc_in = nc.dram_tensor("cc_in", [TPP * P, 512], bf16).ap()
    cc_out = nc.dram_tensor("cc_out", [TPP * P, 512], bf16, addr_space="Shared").ap()

    with tile.TileContext(nc) as tc, ExitStack() as ctx:
        persist = ctx.enter_context(tc.tile_pool(name="persist", bufs=1))
        wpool = ctx.enter_context(tc.tile_pool(name="weights", bufs=1))
        stage = ctx.enter_context(tc.tile_pool(name="stage", bufs=4))
        evac = ctx.enter_context(tc.tile_pool(name="evac", bufs=4))
        pp_big = ctx.enter_context(tc.tile_pool(name="ps_big", bufs=2, space="PSUM"))
        pp_segH = ctx.enter_context(tc.tile_pool(name="ps_segH", bufs=2, space="PSUM"))
        pp_segF = ctx.enter_context(tc.tile_pool(name="ps_segF", bufs=1, space="PSUM"))
        pp_fp = ctx.enter_context(tc.tile_pool(name="ps_fp", bufs=1, space="PSUM"))

        # ---- bulk loads: weights + leaf/crown embedding columns first ----
        wxi = [wpool.tile([P, 768], bf16, tag=f"wxi{i}", name=f"wxi{i}") for i in range(NKT)]
        wxf = [wpool.tile([P, 256], bf16, tag=f"wxf{i}", name=f"wxf{i}") for i in range(NKT)]
        whp = [wpool.tile([P, 768], bf16, tag=f"wh{i}", name=f"wh{i}") for i in range(2)]
        wfhp = [wpool.tile([P, 256], bf16, tag=f"wfh{i}", name=f"wfh{i}") for i in range(2)]
        for i in range(NKT):
            nc.sync.dma_start(wxi[i][:], WxI_d[i * P:(i + 1) * P])
            nc.sync.dma_start(wxf[i][:], WxF_d[i * P:(i + 1) * P])
        for i in range(2):
            nc.sync.dma_start(whp[i][:], Whp_d[i * P:(i + 1) * P])
            nc.sync.dma_start(wfhp[i][:], Wfhp_d[i * P:(i + 1) * P])
        embsT = [wpool.tile([P, NPHA], bf16, tag=f"eT{i}", name=f"eT{i}") for i in range(NKT)]
        embsparT = [wpool.tile([P, NPHA], bf16, tag=f"epT{i}", name=f"epT{i}") for i in range(NKT)]
        for i in range(NKT):  # leaf region + crown columns first
            nc.sync.dma_start(embsT[i][:, 0:NL0], embsT_d[i * P:(i + 1) * P, 0:NL0])
            nc.sync.dma_start(embsparT[i][:, 0:NL0], embsparT_d[i * P:(i + 1) * P, 0:NL0])
        for i in range(NKT):
            nc.sync.dma_start(embsT[i][:, NBOT:NPHA], embsT_d[i * P:(i + 1) * P, NBOT:NPHA])
            nc.sync.dma_start(embsparT[i][:, NBOT:NPHA], embsparT_d[i * P:(i + 1) * P, NBOT:NPHA])
        S_sb = wpool.tile([P, NS * P], bf16, tag="S", name="S")
        nc.sync.dma_start(S_sb[:], S_d[:])
        for i in range(NKT):  # internal region last
            nc.sync.dma_start(embsT[i][:, NL0:NBOT], embsT_d[i * P:(i + 1) * P, NL0:NBOT])
            nc.sync.dma_start(embsparT[i][:, NL0:NBOT], embsparT_d[i * P:(i + 1) * P, NL0:NBOT])

        identf = wpool.tile([P, P], f32, tag="idf", name="idf")
        from concourse.masks import make_identity
        make_identity(nc, identf[:])
        identb = wpool.tile([P, P], bf16, tag="idb", name="idb")
        nc.vector.tensor_copy(identb[:], identf[:])

        contrib = persist.tile([P, NT * 512], bf16, tag="contrib", name="contrib")
        topc = persist.tile([P, TPP * 512], bf16, tag="topc", name="topc")
        nc.gpsimd.memset(topc[:], 0.0)
        ccR = persist.tile([P, TPP * 512], bf16, tag="ccR", name="ccR")
        xfC = [persist.tile([P, 1024], bf16, tag=f"xfC{t}", name=f"xfC{t}") for t in range(TPP)]

        def Stile(sidx, n=P):
            return S_sb[:, sidx * P: sidx * P + n]

        # --------- per-piece pipeline pieces, emitted in 3 stages ---------
        # A "job" is a dict carrying live tiles between stages.

        def stage1(out_sb, tcol, s0, n, srcs, crown_tp=None):
            """seg h-matmuls (+fc old), x-side matmuls, iou, gates, c=i*u.
            srcs: list of (src_tile, sidx, hot) reading out-ish buffers; for
            crown, reads topc and prepends the ccR identity contribution."""
            job = dict(out_sb=out_sb, tcol=tcol, s0=s0, n=n)
            sl = slice(0, n)
            leaf = srcs is None
            big = pp_big.tile([P, 1024], f32, space="PSUM", tag="big", name="big")
            job["big"] = big
            # x-side group A (0:768) and B (768:1024); bf16 rhs -> single mms
            leaf0 = srcs is None
            if crown_tp is None:
                for i in range(NKT):
                    nc.tensor.matmul(big[:, 0:512], lhsT=embsT[i][:, s0:s0 + P],
                                     rhs=wxi[i][:, 0:512], start=(i == 0),
                                     stop=(leaf0 and i == NKT - 1))
                for i in range(NKT):
                    nc.tensor.matmul(big[:, 512:768], lhsT=embsT[i][:, s0:s0 + P],
                                     rhs=wxi[i][:, 512:768], start=(i == 0), stop=False)
                for i in range(NKT):
                    nc.tensor.matmul(big[:, 768:1024], lhsT=embsparT[i][:, s0:s0 + P],
                                     rhs=wxf[i][:], start=False,
                                     stop=(leaf0 and i == NKT - 1))
            else:
                nc.tensor.matmul(big[0:n, 0:512], lhsT=identb[:, 0:n],
                                 rhs=xfC[crown_tp][:, 0:512], start=True, stop=False)
                nc.tensor.matmul(big[0:n, 512:768], lhsT=identb[:, 0:n],
                                 rhs=xfC[crown_tp][:, 512:768], start=True, stop=False)
            if not leaf:
                src_sb = topc if crown_tp is not None else contrib
                segH = pp_segH.tile([P, 512], f32, space="PSUM", tag="segH", name="segH")
                segF = pp_segF.tile([P, 512], f32, space="PSUM", tag="segF", name="segF")
                job["segF"] = segF
                # h chunks: all srcs now. fc: only non-hot now.
                hmms = []
                fcold = []
                fchot = []
                if crown_tp is not None:
                    hmms.append(("cc", None))
                    fcold.append(("cc", None))
                for (t, sidx, hot) in srcs:
                    hmms.append(("s", (t, sidx)))
                    (fchot if hot else fcold).append(("s", (t, sidx)))
                for k, (kind, ts) in enumerate(hmms):
                    for fk in range(2):
                        first = (k == 0 and fk == 0)
                        last = (k == len(hmms) - 1 and fk == 1)
                        if kind == "cc":
                            nc.tensor.matmul(
                                segH[:, fk * P:fk * P + n],
                                lhsT=ccR[:, crown_tp * 512 + fk * P:crown_tp * 512 + (fk + 1) * P],
                                rhs=identb[:, 0:n], start=first, stop=last)
                        else:
                            t, sidx = ts
                            nc.tensor.matmul(
                                segH[:, fk * P:fk * P + n],
                                lhsT=src_sb[:, t * 512 + fk * P:t * 512 + (fk + 1) * P],
                                rhs=Stile(sidx, n), start=first, stop=last)
                for k, (kind, ts) in enumerate(fcold):
                    first = (k == 0)
                    last = (k == len(fcold) - 1) and not fchot
                    if kind == "cc":
                        nc.tensor.matmul(segF[0:n, 0:256], lhsT=identb[:, 0:n],
                                         rhs=ccR[:, crown_tp * 512 + 256:crown_tp * 512 + 512],
                                         start=first, stop=last)
                    else:
                        t, sidx = ts
                        nc.tensor.matmul(segF[0:n, 0:256], lhsT=Stile(sidx, n),
                                         rhs=src_sb[:, t * 512 + 256:t * 512 + 512],
                                         start=first, stop=last)
                job["fchot"] = [(src_sb, ts) for (kind, ts) in fchot]
                job["fc_started"] = bool(fcold)
                # hs^T -> SBUF, iou matmuls close group A
                hsT = evac.tile([P, 256], bf16, tag="hsT", name="hsT")
                if n == P:
                    nc.vector.tensor_copy(hsT[:], segH[:, 0:256])
                else:
                    nc.vector.tensor_copy(hsT[:, 0:n], segH[:, 0:n])
                    nc.vector.tensor_copy(hsT[:, P:P + n], segH[:, P:P + n])
                for i in range(2):
                    nc.tensor.matmul(big[sl, 0:512], lhsT=hsT[:, i * P:i * P + n],
                                     rhs=whp[i][:, 0:512], start=False, stop=(i == 1))
                for i in range(2):
                    nc.tensor.matmul(big[sl, 512:768], lhsT=hsT[:, i * P:i * P + n],
                                     rhs=whp[i][:, 512:768], start=False, stop=(i == 1))
            else:
                job["fchot"] = []
            u_sb = stage.tile([P, 256], bf16, tag="u", name="u_sb")
            io_sb = stage.tile([P, 512], bf16, tag="io", name="io_sb")
            nc.scalar.activation(u_sb[sl, :], big[sl, 512:768], TANH)
            nc.scalar.activation(io_sb[sl, 0:256], big[sl, 0:256], SIG)
            c_sb = stage.tile([P, 256], f32, tag="c", name="c_sb")
            nc.vector.tensor_mul(c_sb[sl, :], io_sb[sl, 0:256], u_sb[sl, :])
            nc.scalar.activation(io_sb[sl, 256:512], big[sl, 256:512], SIG)
            if crown_tp is None:
                fx_sb = stage.tile([P, 256], bf16, tag="fx", name="fx_sb")
                nc.vector.tensor_copy(fx_sb[sl, :], big[sl, 768:1024])
                job["fx_src"] = fx_sb
            else:
                job["fx_src"] = xfC[crown_tp][:, 768:1024]
            job["io"] = io_sb
            job["c"] = c_sb
            job["leaf"] = leaf
            return job

        def stage2(job):
            """c += fc_sum; tanh; h -> out_sb."""
            n = job["n"]
            sl = slice(0, n)
            if not job["leaf"]:
                nc.vector.tensor_add(job["c"][sl, :], job["c"][sl, :],
                                     job["segF"][sl, 0:256])
            th_sb = stage.tile([P, 256], bf16, tag="th", name="th_sb")
            nc.scalar.activation(th_sb[sl, :], job["c"][sl, :], TANH)
            hv = job["out_sb"][:, job["tcol"]:job["tcol"] + 256]
            nc.vector.tensor_mul(hv[sl, :], job["io"][sl, 256:512], th_sb[sl, :])

        def stage3(job, next_jobs):
            """transpose h, wfh, f, fc -> out_sb; then close fc groups of
            next_jobs whose hot sources read this level's fc."""
            n = job["n"]
            sl = slice(0, n)
            big = job["big"]
            hv = job["out_sb"][:, job["tcol"]:job["tcol"] + 256]
            fp = pp_fp.tile([P, 512], f32, space="PSUM", tag="fp", name="fp")
            tt = fp[:, 256:384].bitcast(bf16)
            nc.tensor.transpose(tt[0:P, 0:n], in_=hv[sl, 0:P], identity=identb[sl, sl])
            nc.tensor.transpose(tt[0:P, 128:128 + n], in_=hv[sl, 128:256],
                                identity=identb[sl, sl])
            hT = evac.tile([P, 256], bf16, tag="hT", name="hT")
            if n == P:
                nc.vector.tensor_copy(hT[:], tt[:])
            else:
                nc.vector.tensor_copy(hT[:, 0:n], tt[:, 0:n])
                nc.vector.tensor_copy(hT[:, P:P + n], tt[:, P:P + n])
            for i in range(2):
                nc.tensor.matmul(fp[sl, 0:256], lhsT=hT[:, i * P:i * P + n],
                                 rhs=wfhp[i][:], start=(i == 0), stop=(i == 1))
            ft_sb = stage.tile([P, 256], f32, tag="ft", name="ft_sb")
            nc.vector.tensor_add(ft_sb[sl, :], fp[sl, 0:256], job["fx_src"][sl, :])
            f_sb = stage.tile([P, 256], bf16, tag="f", name="f_sb")
            nc.scalar.activation(f_sb[sl, :], ft_sb[sl, :], SIG)
            nc.vector.tensor_mul(job["out_sb"][sl, job["tcol"] + 256:job["tcol"] + 512],
                                 f_sb[sl, :], job["c"][sl, :])

        def emit_fchot(next_jobs):
            for nj in next_jobs:
                fchot = nj.get("fchot") or []
                for k, (src_sb, (t, sidx)) in enumerate(fchot):
                    nn = nj["n"]
                    nc.tensor.matmul(nj["segF"][0:nn, 0:256], lhsT=Stile(sidx, nn),
                                     rhs=src_sb[:, t * 512 + 256:t * 512 + 512],
                                     start=(not nj["fc_started"]) and k == 0,
                                     stop=(k == len(fchot) - 1))

        # ---------------- leaves (full pipeline per piece) ----------------
        for piece in range(NL0 // P):
            j = stage1(contrib, piece * 512, piece * P, P, None)
            stage2(j)
            stage3(j, [])

        # ---------------- crown phase A -> xfC (early) ----------------
        for t in range(TPP):
            big = pp_big.tile([P, 1024], f32, space="PSUM", tag="big", name="big")
            sA = NBOT + t * P
            for i in range(NKT):
                nc.tensor.matmul(big[:, 0:512], lhsT=embsT[i][:, sA:sA + P],
                                 rhs=wxi[i][:, 0:512], start=(i == 0), stop=(i == NKT - 1))
            for i in range(NKT):
                nc.tensor.matmul(big[:, 512:768], lhsT=embsT[i][:, sA:sA + P],
                                 rhs=wxi[i][:, 512:768], start=(i == 0), stop=False)
            for i in range(NKT):
                nc.tensor.matmul(big[:, 768:1024], lhsT=embsparT[i][:, sA:sA + P],
                                 rhs=wxf[i][:], start=False, stop=(i == NKT - 1))
            nc.vector.tensor_copy(xfC[t][:], big[:])

        # ---------------- bottom levels, software-pipelined ----------------
        bylevel = {}
        for (l, piece, s0, srcs) in meta["pieces_bot"]:
            bylevel.setdefault(l, []).append((piece, s0, srcs))
        prev_jobs = []
        for l in range(1, meta["L0"]):
            jobs = []
            for (piece, s0, srcs) in bylevel[l]:
                srcs2 = [(t, sidx, tile_level[t] == l - 1 and l - 1 >= 1)
                         for (t, sidx) in srcs]
                jobs.append(stage1(contrib, s0 // P * 512, s0, P, srcs2))
            for pj in prev_jobs:
                stage3(pj, jobs)
            emit_fchot(jobs)
            for j in jobs:
                stage2(j)
            prev_jobs = jobs
        for pj in prev_jobs:
            stage3(pj, [])

        # ---------------- cut pre-reduce + collective ----------------
        bytp = {}
        for (tp, t, sidx) in meta["cut_mms"]:
            bytp.setdefault(tp, []).append((t, sidx))
        for tp in range(TPP):
            mlist = bytp.get(tp, [])
            cc_sb = evac.tile([P, 512], bf16, tag="ccsb", name="cc_sb")
            if mlist:
                ccp = pp_segH.tile([P, 512], f32, space="PSUM", tag="segH", name="ccp")
                for k, (t, sidx) in enumerate(mlist):
                    nc.tensor.matmul(ccp[:], lhsT=Stile(sidx),
                                     rhs=contrib[:, t * 512:(t + 1) * 512],
                                     start=(k == 0), stop=(k == len(mlist) - 1))
                nc.vector.tensor_copy(cc_sb[:], ccp[:])
            else:
                nc.gpsimd.memset(cc_sb[:], 0.0)
            nc.sync.dma_start(cc_in[tp * P:(tp + 1) * P], cc_sb[:])
        if sim_no_collective:
            nc.sync.dma_start(cc_out[:], cc_in[:])
        else:
            nc.gpsimd.collective_compute(
                "AllReduce", mybir.AluOpType.add,
                replica_groups=[list(range(N_CORES))],
                ins=[cc_in[:]], outs=[cc_out[:]],
            )
        for tp in range(TPP):
            nc.sync.dma_start(ccR[:, tp * 512:(tp + 1) * 512],
                              cc_out[tp * P:(tp + 1) * P])

        # ---------------- crown levels, software-pipelined ----------------
        prev_jobs = []
        prev_tp = None
        for (l, tp, cnt_ext, srcs) in meta["pieces_crown"]:
            srcs2 = [(st, sidx, prev_tp is not None and st == prev_tp)
                     for (st, sidx) in srcs]
            jobs = [stage1(topc, tp * 512, None, cnt_ext, srcs2, crown_tp=tp)]
            for pj in prev_jobs:
                stage3(pj, jobs)
            emit_fchot(jobs)
            for j in jobs:
                stage2(j)
            prev_jobs = jobs
            prev_tp = tp
        for pj in prev_jobs:
            stage3(pj, [])

        # ---------------- outputs ----------------
        nc.sync.dma_start(contrib_out[:], contrib[:])
        nc.sync.dma_start(topc_out[:], topc[:])

    nc.compile()
    return nc


# ---------------------------------------------------------------- entry point

_CACHE = {}


def _get_program(parent_bytes, in_dim):
    key = (parent_bytes, in_dim)
    if key not in _CACHE:
        parent = np.frombuffer(parent_bytes, dtype=np.int64)
        meta, data = _preprocess(parent)
        nc = _build_program(meta, in_dim)
        _CACHE[key] = (meta, data, nc)
    return _CACHE[key]


def kernel(embs, parent, Wx, bx, Wh, bh, Wfh, bfh):
    from concourse.bass_utils import run_bass_kernel_spmd

    embs = np.asarray(embs, np.float32)
    parent = np.asarray(parent, np.int64)
    meta, data, nc = _get_program(parent.tobytes(), embs.shape[1])
    in_maps = _build_inputs(meta, data, embs,
                            np.asarray(Wx, np.float32), np.asarray(bx, np.float32),
                            np.asarray(Wh, np.float32), np.asarray(bh, np.float32),
                            np.asarray(Wfh, np.float32), np.asarray(bfh, np.float32))
    res = run_bass_kernel_spmd(nc, in_maps, list(range(N_CORES)))
    return _assemble(meta, res.results)


def _assemble(meta, results):
    N = meta["N"]
    NT = meta["NT"]
    h = np.zeros((N, MD), dtype=np.float32)
    for c in range(N_CORES):
        co = np.asarray(results[c]["contrib_out"], np.float32)  # [128, NT*512]
        na = meta["node_at"][c]
        for t in range(NT):
            blk = co[:, t * 512:t * 512 + 256]
            for r in range(P):
                j = na[t * P + r]
                if 0 <= j:
                    h[j] = blk[r]
    to = np.asarray(results[0]["topc_out"], np.float32)
    for j in meta["top_ids"]:
        ts = int(meta["tslot"][j])
        h[j] = to[ts % P, (ts // P) * 512:(ts // P) * 512 + 256]
    return h

